# revision 14
# baseline (speedup 1.0000x reference)
"""SwiGLU expert FFN (DbrxExpertGLU) on 8 Trainium2 NeuronCores.

reference: down = (silu(x @ w1) * (x @ v1)) @ w2.T
  x [T=4096, H=4096], w1/v1/w2 [H=4096, F=14336], all fp32.

Strategy: token-parallel — shard T across the 8 cores (512 rows each),
replicate weights; no collectives. Each core computes everything
transposed so both matmul stages contract over the partition dim:

  phase 1:  gateT/upT [F, m] = w1T/v1T-tiles.T @ xT   (accumulate over H)
            hT = silu(gateT) * upT                     (elementwise, bf16)
  phase 2:  downT [H, m]     = w2T-tiles.T @ hT        (accumulate over F)

Matmuls run in bf16 (PE peak 78.6 TF/s) with fp32 PSUM accumulation.
Host pre-casts weights to bf16 and retiles them so every weight DMA is
a contiguous [128, *] full-partition transfer.

Set KERNEL_TRACE=1 to capture an NTFF profile; the HW exec time lands
in kernel.last_exec_time_ns.
"""

import os

import numpy as np
import ml_dtypes

import concourse.mybir as mybir
from concourse import bacc, bass_utils
from concourse.tile import TileContext

T, H, F = 4096, 4096, 14336
NCORES = 8
M = T // NCORES  # 512 token rows per core
P = 128
KO = H // P  # 32  k-tiles (phase-1 contraction)
FFO = F // P  # 112 f-tiles
HHO = H // P  # 32  output-row tiles (phase-2)
BF16 = mybir.dt.bfloat16
FP32 = mybir.dt.float32

last_exec_time_ns = None
_cache = {}


def _build():
    nc = bacc.Bacc("TRN2", target_bir_lowering=False, debug=False)
    xT_d = nc.dram_tensor("xT", [P, KO * M], BF16, kind="ExternalInput").ap()
    w1_d = nc.dram_tensor("w1t", [FFO, P, KO * P], BF16, kind="ExternalInput").ap()
    v1_d = nc.dram_tensor("v1t", [FFO, P, KO * P], BF16, kind="ExternalInput").ap()
    w2_d = nc.dram_tensor("w2t", [HHO, P, FFO * P], BF16, kind="ExternalInput").ap()
    out_d = nc.dram_tensor("outT", [HHO, P, M], FP32, kind="ExternalOutput").ap()

    with TileContext(nc) as tc:
        with tc.tile_pool(name="hpool", bufs=1) as hpool:
            # hT[ff] tiles live here across both phases:
            # slice [:, ff*M:(ff+1)*M] holds hT rows ff*128..ff*128+127.
            hT = hpool.tile([P, FFO * M], BF16)

            with (
                tc.tile_pool(name="xpool", bufs=1) as xpool,
                tc.tile_pool(name="wpool", bufs=3) as wpool,
                tc.tile_pool(name="pspool", bufs=2, space="PSUM") as pspool,
                tc.tile_pool(name="actpool", bufs=2) as actpool,
                tc.tile_pool(name="warmpool", bufs=1) as warmpool,
                tc.tile_pool(name="warmps", bufs=1, space="PSUM") as warmps,
            ):
                # Warm the PE HAM clock gate (~4us of junk matmuls) while the
                # initial DMAs are in flight; contents are irrelevant.
                warm = warmpool.tile([P, P], BF16)
                warmp = warmps.tile([P, P], FP32)
                nc.sync.dma_start(out=warm[:], in_=xT_d[:, :P])
                for i in range(36):
                    nc.tensor.matmul(
                        warmp[:], warm[:], warm[:],
                        start=(i == 0), stop=(i == 35),
                    )

                # xT resident: xt[ki, ko*M+m] = x[m, ko*128+ki]; chunked, with
                # the first-needed transfers (chunk 0 and the ff=0 weight
                # tiles) issued ahead of the remaining chunks so the first
                # matmul group can start as early as possible.
                xt = xpool.tile([P, KO * M], BF16)
                CH = KO * M // 4
                nc.sync.dma_start(out=xt[:, :CH], in_=xT_d[:, :CH])
                w1_first = wpool.tile([P, KO * P], BF16, tag="w1")
                v1_first = wpool.tile([P, KO * P], BF16, tag="v1")
                nc.sync.dma_start(out=w1_first[:], in_=w1_d[0])
                nc.sync.dma_start(out=v1_first[:], in_=v1_d[0])
                for c4 in range(1, 4):
                    nc.sync.dma_start(
                        out=xt[:, c4 * CH : (c4 + 1) * CH],
                        in_=xT_d[:, c4 * CH : (c4 + 1) * CH],
                    )

                for ff in range(FFO):
                    # w tile: [ki, ko*128+f] = w1[ko*128+ki, ff*128+f]
                    if ff == 0:
                        w1_tile, v1_tile = w1_first, v1_first
                    else:
                        w1_tile = wpool.tile([P, KO * P], BF16, tag="w1")
                        v1_tile = wpool.tile([P, KO * P], BF16, tag="v1")
                        nc.sync.dma_start(out=w1_tile[:], in_=w1_d[ff])
                        nc.sync.dma_start(out=v1_tile[:], in_=v1_d[ff])

                    pg = pspool.tile([P, M], FP32, tag="pg")
                    pu = pspool.tile([P, M], FP32, tag="pu")
                    for ko in range(KO):
                        nc.tensor.matmul(
                            pg[:],
                            w1_tile[:, ko * P : (ko + 1) * P],
                            xt[:, ko * M : (ko + 1) * M],
                            start=(ko == 0),
                            stop=(ko == KO - 1),
                        )
                    for ko in range(KO):
                        nc.tensor.matmul(
                            pu[:],
                            v1_tile[:, ko * P : (ko + 1) * P],
                            xt[:, ko * M : (ko + 1) * M],
                            start=(ko == 0),
                            stop=(ko == KO - 1),
                        )
                    sg = actpool.tile([P, M], FP32, tag="sg")
                    nc.scalar.activation(
                        sg[:], pg[:], mybir.ActivationFunctionType.Silu
                    )
                    nc.vector.tensor_mul(
                        out=hT[:, ff * M : (ff + 1) * M], in0=sg[:], in1=pu[:]
                    )

            with (
                tc.tile_pool(name="w2pool", bufs=2) as w2pool,
                tc.tile_pool(name="ps2", bufs=2, space="PSUM") as ps2,
                tc.tile_pool(name="opool", bufs=2) as opool,
            ):
                for hh in range(HHO):
                    # w2 tile: [ki, ffo*128+f] = w2[hh*128+f, ffo*128+ki]
                    w2_tile = w2pool.tile([P, FFO * P], BF16, tag="w2")
                    nc.sync.dma_start(out=w2_tile[:], in_=w2_d[hh])
                    pd = ps2.tile([P, M], FP32, tag="pd")
                    for ff in range(FFO):
                        nc.tensor.matmul(
                            pd[:],
                            w2_tile[:, ff * P : (ff + 1) * P],
                            hT[:, ff * M : (ff + 1) * M],
                            start=(ff == 0),
                            stop=(ff == FFO - 1),
                        )
                    ot = opool.tile([P, M], FP32, tag="ot")
                    nc.vector.tensor_copy(out=ot[:], in_=pd[:])
                    nc.sync.dma_start(out=out_d[hh], in_=ot[:])
    nc.compile()
    return nc


def _prep_weights(expert_w1, expert_v1, expert_w2):
    bf = ml_dtypes.bfloat16
    # w1t[ffo, ki, ko*P+f] = w1[ko*P+ki, ffo*P+f]
    w1t = np.ascontiguousarray(
        expert_w1.reshape(KO, P, FFO, P).transpose(2, 1, 0, 3).reshape(FFO, P, KO * P)
    ).astype(bf)
    v1t = np.ascontiguousarray(
        expert_v1.reshape(KO, P, FFO, P).transpose(2, 1, 0, 3).reshape(FFO, P, KO * P)
    ).astype(bf)
    # w2t[hho, ki, ffo*P+f] = w2[hho*P+f, ffo*P+ki]
    w2t = np.ascontiguousarray(
        expert_w2.reshape(HHO, P, FFO, P).transpose(0, 3, 2, 1).reshape(HHO, P, FFO * P)
    ).astype(bf)
    return w1t, v1t, w2t


def kernel(x, expert_w1, expert_v1, expert_w2):
    global last_exec_time_ns
    x = np.asarray(x, dtype=np.float32)
    w1t, v1t, w2t = _prep_weights(
        np.asarray(expert_w1, np.float32),
        np.asarray(expert_v1, np.float32),
        np.asarray(expert_w2, np.float32),
    )

    bf = ml_dtypes.bfloat16
    in_maps = []
    for c in range(NCORES):
        xs = x[c * M : (c + 1) * M]  # [M, H]
        # xt[ki, ko*M+m] = xs[m, ko*P+ki]
        xt = np.ascontiguousarray(
            xs.reshape(M, KO, P).transpose(2, 1, 0).reshape(P, KO * M)
        ).astype(bf)
        in_maps.append({"xT": xt, "w1t": w1t, "v1t": v1t, "w2t": w2t})

    if "nc" not in _cache:
        _cache["nc"] = _build()
    nc = _cache["nc"]

    trace = os.environ.get("KERNEL_TRACE", "") == "1"
    if trace:
        _install_ntff_hook()
    res = None
    for attempt in range(3):
        try:
            res = bass_utils.run_bass_kernel_spmd(
                nc, in_maps, core_ids=list(range(NCORES)), trace=trace
            )
            break
        except Exception:
            # The tunneled device occasionally reports a transient
            # "unrecoverable" state left over from a prior session; it
            # clears on retry.
            if attempt == 2:
                raise
            import time

            time.sleep(20)
    last_exec_time_ns = res.exec_time_ns

    # results[c]["outT"] is downT for core c: [HHO, P, M] with
    # outT[hh, j, m] = down[c*M+m, hh*P+j]
    out = np.empty((T, H), np.float32)
    for c in range(NCORES):
        o = res.results[c]["outT"].reshape(H, M)
        out[c * M : (c + 1) * M] = o.T
    return out


def _install_ntff_hook():
    """Wire the axon NTFF profile hook this image's antenv lacks."""
    import importlib.util
    import sys
    import types

    if "antenv.axon_hooks" in sys.modules:
        return
    so_path = "/opt/axon/libaxon_pjrt.so"
    boot = "/root/.axon_site/trn_agent_boot/trn_boot.py"
    if not (os.path.exists(so_path) and os.path.exists(boot)):
        return
    spec = importlib.util.spec_from_file_location("trn_boot_local", boot)
    trn_boot = importlib.util.module_from_spec(spec)
    spec.loader.exec_module(trn_boot)
    hook = trn_boot._ntff_profile_via_ctypes(so_path)
    m = types.ModuleType("antenv.axon_hooks")
    m.get_axon_ntff_profile_hook = lambda: hook
    m.set_axon_ntff_profile_hook = lambda h: None
    sys.modules["antenv.axon_hooks"] = m
